# revision 9
# baseline (speedup 1.0000x reference)
"""Trainium2 Bass kernel for nn_BBBLSTM: LayerNorm -> LSTM(25->128, T=30) -> MLP head.

Sharding: data-parallel, batch 8192 -> 1024 per core across 8 NeuronCores.
Weights replicated. No collectives.

Per-core design (v2):
  - x ships feature-major as xt32 [30, 32, BC] bf16 with rows per t:
    [x (25) ; mu-slot (1, zeros) ; ones (1) ; pad (5, zeros)].  On device it
    lives in ONE SBUF tile xbig [96, 10*BC]: partition group a = t%3 at base
    partition 32*a (PE-legal bases), free block g = t//3.
  - LN stats via PE with batch on the output partitions: per (g, batch-block)
    matmul(lhsT=xbig-slab [96,128], rhs=block-ones [96,3]) -> psum [128,3]
    giving the 3 per-t feature sums for those 128 batch rows; same against
    a squared tile for sum-of-squares.  All 32*2 t-slots for all 8 blocks
    fit one psum bank [128, 512].  mu/rstd math runs on [128, 256] tiles,
    is bounced through DRAM, and scattered back: mu into xbig's mu-slot
    rows, rstd broadcast into a matching in1 tile (rows +0..25 rstd, +26..31
    ones).  One elementwise multiply per block folds the whole LayerNorm:
    xbig <- xbig * in1 = [x*rstd ; mu*rstd ; 1 ; 0].
  - The LSTM gate matmul then needs NO per-step elementwise or DMA:
    psum[gate,b] = sum_f x~*W' - (mu*rstd)*colsum(W') + bias + h @ w_hh
    with stationary w_x3 = [diag(ln_g) w_ih ; -colsum ; bias] replicated at
    partition bases 0/32/64 (g-gate cols pre-doubled for the
    tanh(g) = 2*sigmoid(2g)-1 trick).
  - Per step, 2 batch chunks of 512: 8 matmuls -> sigmoid [128, 2048] ->
    DVE cell math (bf16 2x modes) -> tanh [128,512] -> h update.
  - MLP head on h_last, output [2, BC] -> host transposes.
"""

import ml_dtypes
import numpy as np

BF16 = ml_dtypes.bfloat16

import concourse.bacc as bacc
import concourse.bass as bass
import concourse.mybir as mybir
from concourse.tile import TileContext

B, T, F, H = 8192, 30, 25, 128
NCORES = 8
BC = B // NCORES          # 1024 batch rows per core
G = 4 * H                 # 512 gate width
NB = 512                  # matmul moving free dim / batch chunk
NCHUNK = BC // NB         # 2
FR = 27                   # augmented feature rows: x(25) + mu + ones
GS = 32                   # partition group stride (PE base-partition quantum)
PGRP = 3                  # partition groups (t%3) at bases 0/32/64
NG = 10                   # free blocks (t//3)
TP = 32                   # t slots in the stats tiles
EPS = 1e-5
FP = mybir.dt.float32
BF = mybir.dt.bfloat16
AF = mybir.ActivationFunctionType
OP = mybir.AluOpType

_CACHE = {}


def _build_nc():
    nc = bacc.Bacc()

    xt32 = nc.declare_dram_parameter("xt32", [T, GS, BC], BF, isOutput=False)
    w_x3 = nc.declare_dram_parameter("w_x3", [PGRP * GS, G], BF, isOutput=False)
    w_hh = nc.declare_dram_parameter("w_hh", [H, G], BF, isOutput=False)
    sum_stat = nc.declare_dram_parameter("sum_stat", [PGRP * GS, PGRP], BF, isOutput=False)
    w1 = nc.declare_dram_parameter("w1", [H, H], BF, isOutput=False)
    b1 = nc.declare_dram_parameter("b1", [H, 1], FP, isOutput=False)
    w2 = nc.declare_dram_parameter("w2", [H, H // 2], BF, isOutput=False)
    b2 = nc.declare_dram_parameter("b2", [H // 2, 1], FP, isOutput=False)
    w3 = nc.declare_dram_parameter("w3", [H // 2, 2], BF, isOutput=False)
    b3 = nc.declare_dram_parameter("b3", [2, 1], FP, isOutput=False)
    ones_row = nc.declare_dram_parameter("ones_row", [1, BC], BF, isOutput=False)
    out = nc.declare_dram_parameter("out", [2, BC], FP, isOutput=True)

    # DRAM bounce buffers for the stats scatter
    mu_d = nc.dram_tensor("mu_d", [TP, BC], BF)
    rstd_d = nc.dram_tensor("rstd_d", [TP, BC], BF)

    NROW = PGRP * GS          # 96 partitions in the big tiles
    NF = NG * BC              # 10240 free columns in the big tiles
    NBB = BC // 128           # 8 batch blocks per core

    from contextlib import ExitStack

    with TileContext(nc) as tc, ExitStack() as ctx:
        consts = ctx.enter_context(tc.tile_pool(name="consts", bufs=1))
        state = ctx.enter_context(tc.tile_pool(name="state", bufs=1))
        stp = ctx.enter_context(tc.tile_pool(name="stp", bufs=2))
        xsqp = ctx.enter_context(tc.tile_pool(name="xsqp", bufs=2))
        sigp = ctx.enter_context(tc.tile_pool(name="sigp", bufs=4))
        tmpp = ctx.enter_context(tc.tile_pool(name="tmpp", bufs=4))
        mlpp = ctx.enter_context(tc.tile_pool(name="mlpp", bufs=2))

        # ---- constants into SBUF ----
        w_x_sb = consts.tile([NROW, G], BF)
        nc.gpsimd.dma_start(out=w_x_sb, in_=w_x3[:, :])
        w_hh_sb = consts.tile([H, G], BF)
        nc.gpsimd.dma_start(out=w_hh_sb, in_=w_hh[:, :])
        ss_sb = consts.tile([NROW, PGRP], BF)
        nc.gpsimd.dma_start(out=ss_sb, in_=sum_stat[:, :])
        w1_sb = consts.tile([H, H], BF)
        nc.gpsimd.dma_start(out=w1_sb, in_=w1[:, :])
        b1_sb = consts.tile([H, 1], FP)
        nc.gpsimd.dma_start(out=b1_sb, in_=b1[:, :])
        w2_sb = consts.tile([H, H // 2], BF)
        nc.gpsimd.dma_start(out=w2_sb, in_=w2[:, :])
        b2_sb = consts.tile([H // 2, 1], FP)
        nc.gpsimd.dma_start(out=b2_sb, in_=b2[:, :])
        w3_sb = consts.tile([H // 2, 2], BF)
        nc.gpsimd.dma_start(out=w3_sb, in_=w3[:, :])
        b3_sb = consts.tile([2, 1], FP)
        nc.gpsimd.dma_start(out=b3_sb, in_=b3[:, :])
        eps_sb = consts.tile([H, 1], FP)
        nc.vector.memset(eps_sb, EPS)

        # identity matrix for PE-mode transpose
        id_i = consts.tile([128, 128], mybir.dt.int32)
        nc.gpsimd.iota(id_i, pattern=[[1, 128]], base=0, channel_multiplier=-1)
        id_f = consts.tile([128, 128], FP)
        nc.vector.tensor_scalar(out=id_f, in0=id_i, scalar1=0, scalar2=None,
                                op0=OP.is_equal)

        # ---- the big resident tiles ----
        xbig = state.tile([NROW, NF], BF)    # x rows -> later x~ = x*rstd
        in1 = state.tile([NROW, NF], BF)     # rstd broadcast + ones/pad rows

        # load xt32 into xbig: per partition group a, all 10 g-blocks at once
        for a in range(PGRP):
            src = bass.AP(
                tensor=xt32, offset=a * GS * BC,
                ap=[[BC, GS], [PGRP * GS * BC, NG], [1, BC]])
            nc.gpsimd.dma_start(out=xbig[a * GS:(a + 1) * GS, :], in_=src)

        # ---- phase 0: LN stats ----
        # psum layout [128, 512] fp32: free = 64*bb + 32*kind + t, t = 3g+a
        ps0_cm = tc.tile_pool(name="ps0", bufs=1, space="PSUM")
        ps0 = ps0_cm.__enter__()
        ps_st = ps0.tile([128, 512], FP, tag="st")
        nc.vector.memset(ps_st, 0.0)

        for g in range(NG):
            gs = slice(g * BC, (g + 1) * BC)
            xsq = xsqp.tile([NROW, BC], BF, tag=f"xsq{g % 2}")
            nc.vector.tensor_mul(xsq, xbig[:, gs], xbig[:, gs])
            for bb in range(NBB):
                sl = slice(g * BC + bb * 128, g * BC + (bb + 1) * 128)
                nc.tensor.matmul(
                    ps_st[:, 64 * bb + 3 * g:64 * bb + 3 * g + 3],
                    xbig[:, sl], ss_sb[:, :], start=True, stop=True)
                nc.tensor.matmul(
                    ps_st[:, 64 * bb + 32 + 3 * g:64 * bb + 32 + 3 * g + 3],
                    xsq[:, bb * 128:(bb + 1) * 128], ss_sb[:, :],
                    start=True, stop=True)

        # mu = sum/25 (bf16), var = ssq/25 - mu^2, rstd = 1/sqrt(var+eps)
        # views: [128, bb, kind, t]
        psv = ps_st[:, :].rearrange("p (b k t) -> p b k t", b=NBB, k=2, t=TP)
        mu_f = stp.tile([128, NBB * TP], FP, tag="mu")
        muv = mu_f[:, :].rearrange("p (b t) -> p b t", b=NBB, t=TP)
        nc.vector.tensor_scalar_mul(muv, psv[:, :, 0, :], 1.0 / F)
        mu2 = stp.tile([128, NBB * TP], FP, tag="mu2")
        mu2v = mu2[:, :].rearrange("p (b t) -> p b t", b=NBB, t=TP)
        nc.vector.tensor_mul(mu2, mu_f, mu_f)
        var = stp.tile([128, NBB * TP], FP, tag="var")
        varv = var[:, :].rearrange("p (b t) -> p b t", b=NBB, t=TP)
        nc.vector.scalar_tensor_tensor(
            out=varv, in0=psv[:, :, 1, :], scalar=1.0 / F, in1=mu2v,
            op0=OP.mult, op1=OP.subtract)
        sd = stp.tile([128, NBB * TP], FP, tag="sd")
        nc.scalar.activation(sd, var, AF.Sqrt, bias=eps_sb[:, 0:1])
        rstd_f = stp.tile([128, NBB * TP], FP, tag="rstd")
        nc.vector.reciprocal(rstd_f, sd)

        # transpose stats on the PE: [128 p, slot] -> [slot, p], slot = 32*b'+t
        muT = stp.tile([128, 2 * 128], BF, tag="muT")
        rstdT = stp.tile([128, 2 * 128], BF, tag="rstdT")
        for dst_t, src_t in ((muT, mu_f), (rstdT, rstd_f)):
            for kk in range(2):
                tr = ps0.tile([128, 128], FP, tag=f"tr{kk}")
                nc.tensor.transpose(tr, src_t[:, kk * 128:(kk + 1) * 128], id_f)
                nc.vector.tensor_copy(dst_t[:, kk * 128:(kk + 1) * 128], tr)
        ps0_cm.__exit__(None, None, None)

        # bounce stats to DRAM [t, b] with b = 128*beta + p; beta = 4*kk + q
        for name_t, src_t in ((mu_d, muT), (rstd_d, rstdT)):
            for q in range(4):
                src = src_t[32 * q:32 * q + 32, :].rearrange(
                    "s (k p) -> s k p", k=2, p=128)
                dst = bass.AP(tensor=name_t, offset=128 * q,
                              ap=[[BC, TP], [512, 2], [1, 128]])
                nc.gpsimd.dma_start(out=dst, in_=src)

        # scatter into xbig mu-rows and in1
        for a in range(PGRP):
            # mu row (partition a*GS + 25): mu[t=3g+a, :] per free block g
            src = bass.AP(tensor=mu_d, offset=a * BC,
                          ap=[[PGRP * BC, NG], [1, BC]])
            nc.gpsimd.dma_start(out=xbig[a * GS + F:a * GS + F + 1, :], in_=src)
            # rstd broadcast rows +0..25
            src = bass.AP(tensor=rstd_d, offset=a * BC,
                          ap=[[0, F + 1], [PGRP * BC, NG], [1, BC]])
            nc.gpsimd.dma_start(out=in1[a * GS:a * GS + F + 1, :], in_=src)
            # ones rows +26..31 (ones row + pads)
            src = bass.AP(tensor=ones_row, offset=0,
                          ap=[[0, GS - F - 1], [0, NG], [1, BC]])
            nc.gpsimd.dma_start(out=in1[a * GS + F + 1:(a + 1) * GS, :], in_=src)

        # prescale: xbig <- xbig * in1  (per g block so the recurrence can start)
        for g in range(NG):
            gs = slice(g * BC, (g + 1) * BC)
            nc.vector.tensor_mul(xbig[:, gs], xbig[:, gs], in1[:, gs])

        # ---- phase 1: recurrence ----
        h = state.tile([H, BC], BF)
        c = state.tile([H, BC], BF)
        tc_t = state.tile([H, BC], BF)

        ps1_cm = tc.tile_pool(name="ps1", bufs=1, space="PSUM")
        ps1 = ps1_cm.__enter__()
        for t in range(T):
            a, g = t % PGRP, t // PGRP
            mrows = slice(a * GS, a * GS + FR)
            for cc in range(NCHUNK):
                S = slice(cc * NB, (cc + 1) * NB)
                ms = slice(g * BC + cc * NB, g * BC + (cc + 1) * NB)
                psI = ps1.tile([128, 4 * NB], FP, tag=f"ps{cc}")
                for k in range(4):
                    d = psI[:, k * NB:(k + 1) * NB]
                    wsl = slice(k * H, (k + 1) * H)
                    if t == 0:
                        nc.tensor.matmul(d, w_x_sb[mrows, wsl], xbig[mrows, ms],
                                         start=True, stop=True)
                    else:
                        nc.tensor.matmul(d, w_x_sb[mrows, wsl], xbig[mrows, ms],
                                         start=True, stop=False)
                        nc.tensor.matmul(d, w_hh_sb[:, wsl], h[:, S],
                                         start=False, stop=True)
                sg = sigp.tile([128, 4 * NB], BF, tag=f"sg{cc}")
                nc.scalar.activation(sg, psI, AF.Sigmoid)

                # cell math for this chunk
                i_ = sg[:, 0:NB]
                f_ = sg[:, NB:2 * NB]
                g_ = sg[:, 2 * NB:3 * NB]
                o_ = sg[:, 3 * NB:4 * NB]
                tmp = tmpp.tile([128, NB], BF, tag=f"tmp{cc}")
                nc.vector.tensor_mul(tmp, i_, g_)
                if t == 0:
                    # c = i*tanh(g) = 2*i*sg - i
                    nc.vector.scalar_tensor_tensor(
                        out=c[:, S], in0=tmp, scalar=2.0, in1=i_,
                        op0=OP.mult, op1=OP.subtract)
                else:
                    u = tmpp.tile([128, NB], BF, tag=f"u{cc}")
                    nc.vector.scalar_tensor_tensor(
                        out=u, in0=tmp, scalar=2.0, in1=i_,
                        op0=OP.mult, op1=OP.subtract)
                    nc.vector.tensor_mul(c[:, S], f_, c[:, S])
                    nc.vector.tensor_add(c[:, S], c[:, S], u)
                nc.scalar.activation(tc_t[:, S], c[:, S], AF.Tanh)
                nc.vector.tensor_mul(h[:, S], o_, tc_t[:, S])
        ps1_cm.__exit__(None, None, None)

        # ---- phase 2: MLP head ----
        ps2_cm = tc.tile_pool(name="ps2", bufs=2, space="PSUM")
        ps2 = ps2_cm.__enter__()
        for cc in range(NCHUNK):
            S = slice(cc * NB, (cc + 1) * NB)
            ps1m = ps2.tile([H, NB], FP, tag="m")
            nc.tensor.matmul(ps1m, w1_sb, h[:, S], start=True, stop=True)
            y1 = mlpp.tile([H, NB], BF, tag="y1")
            nc.scalar.activation(y1, ps1m, AF.Relu, bias=b1_sb[:, 0:1])
            ps2m = ps2.tile([H // 2, NB], FP, tag="m")
            nc.tensor.matmul(ps2m, w2_sb, y1, start=True, stop=True)
            y2 = mlpp.tile([H // 2, NB], BF, tag="y2")
            nc.scalar.activation(y2, ps2m, AF.Relu, bias=b2_sb[:, 0:1])
            ps3 = ps2.tile([2, NB], FP, tag="m")
            nc.tensor.matmul(ps3, w3_sb, y2, start=True, stop=True)
            y3 = mlpp.tile([2, NB], FP, tag="y3")
            nc.vector.tensor_scalar_add(y3, ps3, b3_sb[:, 0:1])
            nc.sync.dma_start(out=out[:, S], in_=y3)
        ps2_cm.__exit__(None, None, None)

    nc.finalize()
    return nc


def _get_nc():
    if "nc" not in _CACHE:
        _CACHE["nc"] = _build_nc()
    return _CACHE["nc"]


def _make_in_maps(x, ln_gamma, ln_beta, w_ih, w_hh, b_lstm, w1, b1, w2, b2, w3, b3):
    f32 = np.float32
    x = np.asarray(x, f32)
    ln_gamma = np.asarray(ln_gamma, f32)
    ln_beta = np.asarray(ln_beta, f32)
    w_ih = np.asarray(w_ih, f32)

    Wp = ln_gamma[:, None] * w_ih                      # (25, 512)
    s1 = Wp.sum(0)                                     # (512,)
    bias = np.asarray(b_lstm, f32) + ln_beta @ w_ih    # (512,)
    g2 = np.ones(G, f32)
    g2[2 * H:3 * H] = 2.0                              # g-gate: sigmoid(2x) trick
    w_x = np.concatenate([Wp * g2, (-s1 * g2)[None], (bias * g2)[None]], 0)  # (27, 512)
    w_x3 = np.zeros((PGRP * GS, G), f32)
    for a in range(PGRP):
        w_x3[a * GS:a * GS + FR] = w_x
    whh = np.asarray(w_hh, f32) * g2

    # block-ones moving operand for the stats sums: col a selects the x rows
    # of partition group a
    ss = np.zeros((PGRP * GS, PGRP), f32)
    for a in range(PGRP):
        ss[a * GS:a * GS + F, a] = 1.0

    shared = {
        "w_x3": np.ascontiguousarray(w_x3).astype(BF16),
        "w_hh": np.ascontiguousarray(whh).astype(BF16),
        "sum_stat": np.ascontiguousarray(ss).astype(BF16),
        "w1": np.ascontiguousarray(w1, f32).astype(BF16),
        "b1": np.asarray(b1, f32).reshape(H, 1).copy(),
        "w2": np.ascontiguousarray(w2, f32).astype(BF16),
        "b2": np.asarray(b2, f32).reshape(H // 2, 1).copy(),
        "w3": np.ascontiguousarray(w3, f32).astype(BF16),
        "b3": np.asarray(b3, f32).reshape(2, 1).copy(),
        "ones_row": np.ones((1, BC), BF16),
    }
    in_maps = []
    for i in range(NCORES):
        xs = np.asarray(x[i * BC:(i + 1) * BC], f32)   # (BC, T, F)
        xt = xs.transpose(1, 2, 0)                     # (T, F, BC)
        x32 = np.zeros((T, GS, BC), f32)
        x32[:, :F, :] = xt
        x32[:, F + 1, :] = 1.0                         # ones row; mu slot stays 0
        m = dict(shared)
        m["xt32"] = np.ascontiguousarray(x32).astype(BF16)
        in_maps.append(m)
    return in_maps


def _run(in_maps, **kw):
    from concourse.bass_utils import run_bass_kernel_spmd
    nc = _get_nc()
    res = run_bass_kernel_spmd(nc, in_maps, core_ids=list(range(NCORES)), **kw)
    _CACHE["last_results"] = res
    y = np.concatenate([np.asarray(r["out"]).T for r in res.results], axis=0)
    return np.ascontiguousarray(y, np.float32)


def kernel(**inputs):
    return _run(_make_in_maps(**inputs))
